# revision 31
# baseline (speedup 1.0000x reference)
"""AttentionLSTM Trainium2 kernel.

Strategy: data-parallel over batch N=256 across 8 NeuronCores (32 batch each).
Weights replicated per core (fp16 in SBUF, fp32 PSUM accumulation). The
sequential T=64 loop is fully unrolled per core; zero cross-core
communication.

Per-core layouts (partition dim first, 128 partitions):
  h16   : (128 hc, 8 ht, 32 n)  fp16   hidden state, h index = ht*128+hc
  c32   : (128 hc, 8 ht, 32 n)  fp32   cell state
  A1    : (128 hc, 8 ht, 32 n, 16 p) fp16   A_flat transposed
  W_sb  : (128 k, 16 kt, 32 mt, 128 m) fp16  [Wh; Wattn] as lhsT tiles
  gates : PSUM (128 m, 32 mt, 32 n) fp32; m-tiles 0-7=i, 8-15=f, 16-23=o, 24-31=g

Gates matmul is weight-stationary: out[mt] += W[kt,mt].T @ rhs_kt where
rhs is h16 (kt 0-7) then attn16 (kt 8-15).

Attention: scores via DVE broadcast-multiply then PE ones-reduction over
the h partition axis (result replicated over 128 partitions), softmax on
DVE/ACT (max-free exp: |scores/32| is small), attention-weighted sum of A
back on DVE.

x @ Wx + b is precomputed for all t into DRAM (xwxb) at the start, then
streamed per step.
"""

import numpy as np

N = 256
T = 64
D = 1024
H = 1024
NCORES = 8
NB = N // NCORES  # 32 batch per core
HT = H // 128  # 8 h-tiles
P16 = 16
MT = 4 * H // 128  # 32 gate-col tiles
KT = 16  # contraction tiles for [h; attn]

_BUILD_CACHE = {}

# ablation knobs (timing experiments only — wrong math when not defaults)
ABL_GATES_KT = 8   # how many of the 8 k-tiles per part to emit
ABL_NO_ATTN = 0    # 1: skip attention chain, attn16 := h16 copy
ABL_NO_STEP_DMA = 0  # 1: skip per-step xwx load + hs_out store


def _build_program(t_steps=T, repeat=1):
    import concourse.bass as bass
    import concourse.tile as tile
    from concourse import bacc, mybir

    f16 = mybir.dt.float16
    f32 = mybir.dt.float32
    AF = mybir.ActivationFunctionType

    nc = bacc.Bacc("TRN2", target_bir_lowering=False, debug=False,
                   num_devices=NCORES)

    xT16 = nc.dram_tensor("xT16", (D, T, NB), f16, kind="ExternalInput").ap()
    A1d = nc.dram_tensor("A1d", (128, HT, NB, P16), f16,
                         kind="ExternalInput").ap()
    Wf16 = nc.dram_tensor("Wf16", (2 * H, 4 * H), f16,
                          kind="ExternalInput").ap()
    Wx16 = nc.dram_tensor("Wx16", (D, 4 * H), f16, kind="ExternalInput").ap()
    bvec = nc.dram_tensor("bvec", (128, MT), f32, kind="ExternalInput").ap()
    hs_out = nc.dram_tensor("hs_out", (T, 128, HT, NB), f16,
                            kind="ExternalOutput").ap()

    with tile.TileContext(nc, trace_sim=False) as tc:
        _emit(tc, bass, mybir, f16, f32, AF, xT16, A1d, Wf16, Wx16, bvec,
              hs_out, t_steps, repeat)

    nc.compile()
    return nc


def _emit(tc, bass, mybir, f16, f32, AF, xT16, A1d, Wf16, Wx16, bvec, hs_out,
          t_steps, repeat=1):
    from contextlib import ExitStack
    ctx = ExitStack()
    nc = tc.nc
    Alu = mybir.AluOpType
    Ax = mybir.AxisListType

    consts = ctx.enter_context(tc.tile_pool(name="consts", bufs=1))
    dram = ctx.enter_context(tc.tile_pool(name="dram", bufs=1, space="DRAM"))

    # DRAM scratch for precomputed x@Wx + b, layout [t][hc][mt][n]
    xwxb = dram.tile([T, 128, MT, NB], f32)

    A1_sb = consts.tile([128, HT, NB, P16], f16)
    nc.sync.dma_start(A1_sb[:], A1d)
    b_sb = consts.tile([128, MT], f32)
    nc.sync.dma_start(b_sb[:], bvec)
    ones_sb = consts.tile([128, 128], f16)
    nc.vector.memset(ones_sb[:], 1.0)

    # big weights load emitted early so the DMA overlaps the precompute
    W_sb = consts.tile([128, KT, MT, 128], f16)
    nc.sync.dma_start(
        W_sb[:], Wf16.rearrange("(kt p) (mt m) -> p kt mt m", p=128, m=128))

    # ---------------- phase 1: precompute xwxb = x @ Wx + b ----------------
    TCH = 4  # (t,n) chunks of 512 per m-tile
    TPC = T // TCH  # 16 timesteps per chunk
    with tc.tile_pool(name="xt", bufs=1) as xtp, \
         tc.tile_pool(name="wx", bufs=16) as wxp, \
         tc.tile_pool(name="prestage", bufs=3) as stp, \
         tc.tile_pool(name="prepsum", bufs=2, space="PSUM") as ppp:
        xT_sb = xtp.tile([128, HT, T, NB], f16)
        nc.sync.dma_start(
            xT_sb[:], xT16.rearrange("(kt p) t n -> p kt t n", p=128))
        for mt in range(MT):
            wxt = []
            for kt in range(HT):
                w = wxp.tile([128, 128], f16, tag="wx")
                nc.sync.dma_start(
                    w[:], Wx16[kt * 128:(kt + 1) * 128,
                               mt * 128:(mt + 1) * 128])
                wxt.append(w)
            for ch in range(TCH):
                ps = ppp.tile([128, TPC, NB], mybir.dt.float32)
                for kt in range(HT):
                    nc.tensor.matmul(
                        ps[:], wxt[kt][:],
                        xT_sb[:, kt, ch * TPC:(ch + 1) * TPC, :],
                        start=(kt == 0), stop=(kt == HT - 1))
                st = stp.tile([128, TPC, NB], mybir.dt.float32)
                nc.scalar.activation(st[:], ps[:], AF.Identity,
                                     bias=b_sb[:, mt:mt + 1])
                nc.sync.dma_start(
                    xwxb[ch * TPC:(ch + 1) * TPC, :, mt, :].transpose(
                        [1, 0, 2]), st[:])

    # ---------------- phase 2: init state ----------------
    state = ctx.enter_context(tc.tile_pool(name="state", bufs=2))
    work = ctx.enter_context(tc.tile_pool(name="work", bufs=2))
    cell = ctx.enter_context(tc.tile_pool(name="cell", bufs=1))
    xwxp = ctx.enter_context(tc.tile_pool(name="xwx", bufs=2))
    outp = ctx.enter_context(tc.tile_pool(name="outs", bufs=2))
    psg = ctx.enter_context(tc.tile_pool(name="psg", bufs=1, space="PSUM"))
    pss = ctx.enter_context(tc.tile_pool(name="pss", bufs=2, space="PSUM"))

    inv_sqrt_h = 1.0 / np.sqrt(H)

    for _rep in range(repeat):
      # h0 = c0 = mean(A, axis p)
      rs = work.tile([128, HT, NB], mybir.dt.float32, tag="rs")
      nc.vector.tensor_reduce(rs[:], A1_sb[:], Ax.X, Alu.add)
      c32 = state.tile([128, HT, NB], mybir.dt.float32, tag="c")
      nc.scalar.mul(c32[:], rs[:], 1.0 / P16)
      h16 = state.tile([128, HT, NB], f16, tag="h")
      nc.scalar.mul(h16[:], rs[:], 1.0 / P16)

      for t in range(t_steps):
        # -- stream in x@Wx+b for this step
        if not ABL_NO_STEP_DMA:
            xwx_t = xwxp.tile([128, MT, NB], mybir.dt.float32, tag="xwx")
            nc.sync.dma_start(xwx_t[:], xwxb[t])

        ps_s = pss.tile([128, NB, P16], mybir.dt.float32)
        ps_h = psg.tile([128, MT, NB], mybir.dt.float32, tag="ph")
        ps_a = psg.tile([128, MT, NB], mybir.dt.float32, tag="pa")

        if not ABL_NO_ATTN:
            # -- attention scores: tmp = A1 * h (broadcast over p)
            tmp = work.tile([128, HT, NB, P16], f16, tag="tmp")
            nc.vector.tensor_mul(
                tmp[:], A1_sb[:],
                h16[:].unsqueeze(3).broadcast_to((128, HT, NB, P16)))

        def h_part(mt):
            for kt in range(ABL_GATES_KT):
                nc.tensor.matmul(ps_h[:, mt, :], W_sb[:, kt, mt, :],
                                 h16[:, kt, :], start=(kt == 0),
                                 stop=(kt == ABL_GATES_KT - 1))

        for mt in range(10):
            h_part(mt)

        if not ABL_NO_ATTN:
            # scores reduction over h on PE (replicated over partitions)
            for ht in range(HT):
                nc.tensor.matmul(ps_s[:], ones_sb[:], tmp[:, ht, :, :],
                                 start=(ht == 0), stop=(ht == HT - 1))

        for mt in range(10, MT):
            h_part(mt)

        # merge h-part + x@Wx+b while attention chain runs
        pre = cell.tile([128, MT, NB], mybir.dt.float32, tag="pre")
        if ABL_NO_STEP_DMA:
            nc.vector.tensor_copy(pre[:], ps_h[:])
        else:
            nc.vector.tensor_add(pre[:], ps_h[:], xwx_t[:])

        if not ABL_NO_ATTN:
            # -- softmax over p (max-free; scores/sqrt(H) is O(1))
            expv = work.tile([128, NB, P16], f16, tag="expv")
            nc.scalar.activation(expv[:], ps_s[:], AF.Exp,
                                 scale=float(inv_sqrt_h))
            sums = work.tile([128, NB], mybir.dt.float32, tag="sums")
            nc.vector.tensor_reduce(sums[:], expv[:], Ax.X, Alu.add)
            rsum = work.tile([128, NB], mybir.dt.float32, tag="rsum")
            nc.vector.reciprocal(rsum[:], sums[:])
            w16 = work.tile([128, NB, P16], f16, tag="w16")
            nc.vector.tensor_mul(
                w16[:], expv[:],
                rsum[:].unsqueeze(2).broadcast_to((128, NB, P16)))

            # -- attn = sum_p A1 * w
            tmp2 = work.tile([128, HT, NB, P16], f16, tag="tmp")
            nc.vector.tensor_mul(
                tmp2[:], A1_sb[:],
                w16[:].unsqueeze(1).broadcast_to((128, HT, NB, P16)))
            attn16 = work.tile([128, HT, NB], f16, tag="attn16")
            with nc.allow_low_precision(reason="DVE reduces in fp32 internally"):
                nc.vector.tensor_reduce(attn16[:], tmp2[:], Ax.X, Alu.add)
        else:
            attn16 = work.tile([128, HT, NB], f16, tag="attn16")
            nc.vector.tensor_copy(attn16[:], h16[:])

        # -- attn-part matmuls
        for mt in range(MT):
            for kt in range(ABL_GATES_KT):
                nc.tensor.matmul(ps_a[:, mt, :], W_sb[:, HT + kt, mt, :],
                                 attn16[:, kt, :], start=(kt == 0),
                                 stop=(kt == ABL_GATES_KT - 1))

        # -- LSTM cell; per-gate merge so ACT pipelines with the adds
        sIFO = cell.tile([128, 3 * HT, NB], mybir.dt.float32, tag="sIFO")
        tG = cell.tile([128, HT, NB], mybir.dt.float32, tag="tG")
        for g in range(4):
            gs = slice(g * HT, (g + 1) * HT)
            nc.vector.tensor_add(pre[:, gs, :], pre[:, gs, :], ps_a[:, gs, :])
            if g < 3:
                nc.scalar.activation(sIFO[:, gs, :], pre[:, gs, :],
                                     AF.Sigmoid)
            else:
                nc.scalar.activation(tG[:], pre[:, gs, :], AF.Tanh)
        sI = sIFO[:, 0:HT, :]
        sF = sIFO[:, HT:2 * HT, :]
        sO = sIFO[:, 2 * HT:3 * HT, :]

        t1 = cell.tile([128, HT, NB], mybir.dt.float32, tag="t1")
        nc.vector.tensor_mul(t1[:], sF, c32[:])
        t2 = cell.tile([128, HT, NB], mybir.dt.float32, tag="t2")
        nc.vector.tensor_mul(t2[:], sI, tG[:])
        c32 = state.tile([128, HT, NB], mybir.dt.float32, tag="c")
        nc.vector.tensor_add(c32[:], t1[:], t2[:])
        thc = cell.tile([128, HT, NB], mybir.dt.float32, tag="thc")
        nc.scalar.activation(thc[:], c32[:], AF.Tanh)
        h16 = state.tile([128, HT, NB], f16, tag="h")
        nc.vector.tensor_mul(h16[:], sO, thc[:])

        if not ABL_NO_STEP_DMA:
            nc.sync.dma_start(hs_out[t], h16[:])

    ctx.close()


def _prep_inputs(x, A, Wx, Wh, Wattn, b):
    """Host-side sharding + layout transforms. Returns in_maps for 8 cores."""
    Wf16 = np.ascontiguousarray(
        np.concatenate([Wh, Wattn], axis=0).astype(np.float16))
    Wx16 = np.ascontiguousarray(Wx.astype(np.float16))
    b_sb = np.ascontiguousarray(
        b.astype(np.float32).reshape(MT, 128).T)
    in_maps = []
    for c in range(NCORES):
        sl = slice(c * NB, (c + 1) * NB)
        xc = x[sl]  # (NB, T, D)
        xT16 = np.ascontiguousarray(xc.transpose(2, 1, 0).astype(np.float16))
        Ac = A[sl].reshape(NB, H, P16)
        A1 = np.ascontiguousarray(
            Ac.transpose(1, 0, 2).reshape(HT, 128, NB, P16)
            .transpose(1, 0, 2, 3).astype(np.float16))
        in_maps.append({
            "xT16": xT16,
            "A1d": A1,
            "Wf16": Wf16,
            "Wx16": Wx16,
            "bvec": b_sb,
        })
    return in_maps


def kernel(x, A, Wx, Wh, Wattn, b, _trace=False, _t_steps=T):
    x = np.asarray(x, dtype=np.float32)
    A = np.asarray(A, dtype=np.float32)
    Wx = np.asarray(Wx, dtype=np.float32)
    Wh = np.asarray(Wh, dtype=np.float32)
    Wattn = np.asarray(Wattn, dtype=np.float32)
    b = np.asarray(b, dtype=np.float32)

    from concourse.bass_utils import run_bass_kernel_spmd

    key = _t_steps
    if key not in _BUILD_CACHE:
        _BUILD_CACHE[key] = _build_program(_t_steps)
    nc = _BUILD_CACHE[key]

    in_maps = _prep_inputs(x, A, Wx, Wh, Wattn, b)
    res = run_bass_kernel_spmd(nc, in_maps, core_ids=list(range(NCORES)),
                               trace=_trace)

    out = np.empty((N, T, H), dtype=np.float32)
    for c in range(NCORES):
        oc = res.results[c]["hs_out"].astype(np.float32)  # (T, 128, HT, NB)
        out[c * NB:(c + 1) * NB] = (
            oc.transpose(3, 0, 2, 1).reshape(NB, T, H))
    if _trace:
        kernel._last_results = res
    return out
